# revision 13
# baseline (speedup 1.0000x reference)
"""DGAT attention head on 8 trn2 NeuronCores — sorted branch-split.

Math: with z = C*(hz1_i + hz2_j) + D0 and L1 = leaky(A+B) > 0,
  exp(L1*leaky(z)) = max(exp(L1*z), exp(0.2*L1*z)) = max(a1_i*b1_j, a2_i*b2_j)
two rank-1 fields (log-domain rank-1 per branch).  q = adj (.) max-field,
num_i = sum_j q_ij h_j, den = sum_j q_ij, out = elu(num/den).

Layout trick: sort j globally by hz2 and i globally by hz1 (cores take a
round-robin deal of the sorted i order, so all cores share the same range
tables).  For a 128-j tile g, columns i split into contiguous
[pure-branch-1 | mixed | pure-branch-2] ranges.  Pure ranges need NO
elementwise work at all: the raw fp8 {0,1} adjacency tile is fed directly
to the PE as the moving operand against a beta-scaled [h|1] stationary
table, accumulating into per-branch PSUM accumulators that are combined
with the alpha_i factors in the tail.  Only the mixed columns (~2%) get an
elementwise max(a1*b1, a2*b2) (one custom DVE op) and an adj mask multiply.

Per-core traffic: adjt 8MB fp8 + x 4.5MB f16 -> DMA-bound at ~360 GB/s.
"""

import numpy as np
import ml_dtypes

import concourse.bass as bass
import concourse.bacc as bacc
import concourse.mybir as mybir
import concourse.dve_ops as dve_ops
from concourse.dve_spec import Spec, Src0, Src1, C0, C1, Zero, One, maxx
from concourse.tile import TileContext
from concourse.bass_utils import run_bass_kernel_spmd

F32 = mybir.dt.float32
F16 = mybir.dt.float16
BF16 = mybir.dt.bfloat16
F8 = mybir.dt.float8e5
AF = mybir.ActivationFunctionType
OP = mybir.AluOpType

NCORES = 8
SLOPE = 0.2  # leakyrelu negative slope (fixed in the reference)

TRACE = False
LAST_RESULTS = None
LAST_NC = None


def _leaky(z):
    return z if z >= 0.0 else SLOPE * z


def _finish_register(name, spec):
    from concourse.dve_spec import lower
    from concourse.dve_ops import has_src1
    from concourse.dve_uop import DveOpSpec

    op = dve_ops.DveOp(name, spec, subdim=False, uops_sha={})
    dve_ops.OPS.append(op)
    dve_ops.CUSTOM_DVE_SPECS[name] = spec
    dve_ops._SUB_OPCODE_FOR_NAME[name] = (
        dve_ops._CUSTOM_DVE_ROW_BASE + len(dve_ops.OPS) - 1
    )
    assert dve_ops._SUB_OPCODE_FOR_NAME[name] < 0x20
    for ver in ("v3",):
        pinned = DveOpSpec(
            name=name,
            opcode=dve_ops.get_dve_sub_opcode(name),
            uops=lower(spec, ver=ver),
            rd1_en=has_src1(spec),
        ).sha(ver)
        op.uops_sha[ver] = pinned
        dve_ops._COMPILE_CACHE.pop((name, ver), None)
        op.compile(ver)
    return op


def _register_br_max_op():
    name = "BR_MAX_ANT"
    for op in dve_ops.OPS:
        if op.name == name:
            return op
    spec = Spec(
        body=maxx(Src0 * C0, Src1 * C1),
        reference=lambda in0, in1, s0, s1, imm2: np.maximum(
            in0 * s0, in1 * s1
        ).astype(np.float32),
    )
    return _finish_register(name, spec)


def _register_elu_max_op():
    name = "ELU_MAX_ANT"
    for op in dve_ops.OPS:
        if op.name == name:
            return op
    spec = Spec(
        body=maxx(Src0 * C0, Src1 - One),
        reference=lambda in0, in1, s0, s1, imm2: np.maximum(
            in0 * s0, in1 - 1.0
        ).astype(np.float32),
    )
    return _finish_register(name, spec)


def _build(n, din, dout, rows, kpre, sg, eg, mixmax, c_a1, c_a2, c_b1, c_b2):
    """Build the SPMD Bass program (identical on all cores).

    alpha1 = exp(kpre*hz1 + c_a1), beta1 = exp(kpre*hz2 + c_b1) and the
    0.2-slope (branch-2) variants; sg/eg are the shared per-tile pure-range
    breakpoints on the (sorted) own-i axis.
    """
    assert n % 256 == 0 and rows % 128 == 0 and din % 128 == 0
    ng = n // 128
    mt = 4
    nm = ng // mt
    kc = din // 128
    de = dout + 1
    brmax = _register_br_max_op()
    emx = _register_elu_max_op()

    nc = bacc.Bacc("TRN2", target_bir_lowering=False)
    adjt_d = nc.dram_tensor("adjt", [n, rows], F8, kind="ExternalInput")
    xt_d = nc.dram_tensor("xt", [din, n], F16, kind="ExternalInput")
    xto_d = nc.dram_tensor("xt_own", [din, rows], F16, kind="ExternalInput")
    w_d = nc.dram_tensor("w", [din, dout], F32, kind="ExternalInput")
    a_d = nc.dram_tensor("a", [2 * dout, 1], F32, kind="ExternalInput")
    y_d = nc.dram_tensor("y", [rows, dout], F32, kind="ExternalOutput")

    with TileContext(nc) as tc:
        with (
            tc.tile_pool(name="consts", bufs=1) as consts,
            tc.tile_pool(name="adjp", bufs=12) as adjp,
            tc.tile_pool(name="mixp", bufs=8) as mixp,
            tc.tile_pool(name="tailp", bufs=4) as tailp,
        ):
            from concourse.masks import make_identity

            identity0 = consts.tile([128, 128], F32)
            make_identity(nc, identity0)
            identity = consts.tile([128, 128], F32)
            nc.vector.tensor_copy(identity, identity0)

            ones128 = consts.tile([128, 128], F16)
            nc.vector.memset(ones128, 1.0)
            # moving row that deposits 1.0 into ps column `dout` via a
            # 1-partition matmul (the [h|1] trick)
            onerow = consts.tile([1, dout + 2], F16)
            nc.vector.memset(onerow, 0.0)
            nc.vector.memset(onerow[:, dout : dout + 1], 1.0)
            zrow = consts.tile([128, rows], F8)
            nc.vector.memset(zrow, 0.0)

            # a2 broadcast across partitions (partition-step-0 DMA)
            a_ap = a_d[:, :]
            a1bc = consts.tile([128, dout], F32)
            nc.sync.dma_start(
                out=a1bc,
                in_=bass.AP(tensor=a_ap.tensor, offset=0, ap=[[0, 128], [1, dout]]),
            )
            a2bc = consts.tile([128, dout], F32)
            nc.sync.dma_start(
                out=a2bc,
                in_=bass.AP(
                    tensor=a_ap.tensor, offset=dout, ap=[[0, 128], [1, dout]]
                ),
            )

            # wx_k = [w_k | 0 | w_k@a2] f16; the zero col becomes the ones
            # col of each h-tile (filled by the onerow matmul).
            wx = []
            wxraw = []
            for k in range(kc):
                wxr = consts.tile([128, dout + 2], F32, name=f"wxr{k}")
                nc.sync.dma_start(
                    out=wxr[:, 0:dout], in_=w_d[k * 128 : (k + 1) * 128, :]
                )
                nc.vector.memset(wxr[:, dout : dout + 1], 0.0)
                t2 = consts.tile([128, dout], F32, name=f"wb_t{k}")
                nc.vector.tensor_mul(t2, wxr[:, 0:dout], a2bc)
                nc.vector.reduce_sum(
                    wxr[:, dout + 1 : dout + 2], t2, axis=mybir.AxisListType.X
                )
                wxk = consts.tile([128, dout + 2], F16, name=f"wx{k}")
                nc.vector.tensor_copy(wxk, wxr)
                wx.append(wxk)
                wxraw.append(wxr)
            # w@a1 columns (for own-row hz1 broadcast)
            wa1 = []
            for k in range(kc):
                t1 = consts.tile([128, dout], F32, name=f"wa_t{k}")
                nc.vector.tensor_mul(t1, wxraw[k][:, 0:dout], a1bc)
                wa1k = consts.tile([128, 1], F32, name=f"wa1_{k}")
                nc.vector.reduce_sum(wa1k, t1, axis=mybir.AxisListType.X)
                wa1.append(wa1k)

            # bias columns for the four exp shifts
            ca1col = consts.tile([128, 1], F32)
            nc.vector.memset(ca1col, c_a1)
            ca2col = consts.tile([128, 1], F32)
            nc.vector.memset(ca2col, c_a2)
            cb1col = consts.tile([128, 1], F32)
            nc.vector.memset(cb1col, c_b1)
            cb2col = consts.tile([128, 1], F32)
            nc.vector.memset(cb2col, c_b2)

            alpha1bc = consts.tile([128, rows], BF16)
            alpha2bc = consts.tile([128, rows], BF16)
            hb1 = consts.tile([128, ng, de], BF16)
            hb2 = consts.tile([128, ng, de], BF16)
            hx = consts.tile([128, ng, de], BF16)
            b1cols = consts.tile([128, ng], F32)
            b2cols = consts.tile([128, ng], F32)

            # ---- DMA schedule (device processes in this order) ----
            xtos = []
            with tc.tile_pool(name="xtp", bufs=1) as xtp:
                for k in range(kc):
                    xtok = xtp.tile([128, rows], F16, name=f"xto{k}")
                    nc.sync.dma_start(
                        out=xtok, in_=xto_d[k * 128 : (k + 1) * 128, :]
                    )
                    xtos.append(xtok)
                xts = [
                    xtp.tile([128, n], F16, name=f"xt{k}") for k in range(kc)
                ]
                # interleave xt chunks with adjt megatiles so neither
                # stream starves the other on the (serial) DMA device
                xchunk = 1024
                nxc = n // xchunk
                adjt_r = adjt_d[:, :].rearrange(
                    "(m t p) i -> m p t i", t=mt, p=128
                )
                adjts = []
                for m in range(nm):
                    if m < nxc:
                        c0 = m * xchunk
                        for k in range(kc):
                            nc.sync.dma_start(
                                out=xts[k][:, c0 : c0 + xchunk],
                                in_=xt_d[
                                    k * 128 : (k + 1) * 128, c0 : c0 + xchunk
                                ],
                            )
                    adjt_t = adjp.tile([128, mt * rows], F8)
                    nc.sync.dma_start(
                        out=adjt_t.rearrange("p (t i) -> p t i", t=mt),
                        in_=adjt_r[m],
                    )
                    adjts.append(adjt_t)

                # ---- own-row hz1 -> alpha broadcasts ----
                with tc.tile_pool(name="pshz", bufs=1, space="PSUM") as pshz:
                    hz_ps = pshz.tile([128, rows], F32)
                    for k in range(kc):
                        wa1bc = consts.tile([128, 128], F16, name=f"wa1bc{k}")
                        nc.vector.tensor_scalar_mul(wa1bc, ones128, wa1[k])
                        for n0 in range(0, rows, 512):
                            nc.tensor.matmul(
                                hz_ps[:, n0 : n0 + 512],
                                wa1bc,
                                xtos[k][:, n0 : n0 + 512],
                                start=(k == 0),
                                stop=(k == kc - 1),
                            )
                    nc.scalar.activation(
                        alpha1bc, hz_ps, AF.Exp, bias=ca1col[:, 0:1], scale=kpre
                    )
                    nc.scalar.activation(
                        alpha2bc, hz_ps, AF.Exp, bias=ca2col[:, 0:1], scale=SLOPE * kpre
                    )

                # ---- per-group h tiles + main loop, interleaved ----
                with tc.tile_pool(name="psacc", bufs=1, space="PSUM") as psacc:
                    acc1 = psacc.tile([de, rows], F32)
                    acc2 = psacc.tile([de, rows], F32)
                    acc3 = psacc.tile([de, rows], F32)

                    # a matmul's PSUM write cannot cross a 2KB bank (512
                    # f32 cols) -> chunk every column range at 512 bounds
                    def spans(lo, hi):
                        while lo < hi:
                            nxt = min(hi, (lo // 512 + 1) * 512)
                            yield lo, nxt
                            lo = nxt

                    def mm(acc, lhs, rhs, lo, hi, start=False, stop=False):
                        for c0, c1 in spans(lo, hi):
                            nc.tensor.matmul(
                                acc[:, c0:c1],
                                lhs,
                                rhs[:, c0:c1],
                                start=start,
                                stop=stop,
                            )

                    with tc.tile_pool(name="hps", bufs=2, space="PSUM") as hps:

                        def emit_hbuild(g):
                            ps = hps.tile([128, dout + 2], F32, name="ps_h")
                            for k in range(kc):
                                nc.tensor.matmul(
                                    ps,
                                    xts[k][:, g * 128 : (g + 1) * 128],
                                    wx[k],
                                    start=(k == 0),
                                    stop=False,
                                )
                            nc.tensor.matmul(
                                ps, ones128[0:1, :], onerow,
                                start=False, stop=True,
                            )
                            # beta cols (f32, used as scalar ptrs)
                            nc.scalar.activation(
                                b1cols[:, g : g + 1],
                                ps[:, dout + 1 : dout + 2],
                                AF.Exp, bias=cb1col[:, 0:1], scale=kpre,
                            )
                            nc.scalar.activation(
                                b2cols[:, g : g + 1],
                                ps[:, dout + 1 : dout + 2],
                                AF.Exp, bias=cb2col[:, 0:1],
                                scale=SLOPE * kpre,
                            )
                            # raw [h|1] (ACT, bf16) then beta-scaled copies
                            # (DVE TSPs read the SBUF bf16 -> 4x mode)
                            nc.scalar.copy(hx[:, g, :], ps[:, 0:de])
                            nc.vector.tensor_scalar_mul(
                                hb1[:, g, :], hx[:, g, :], b1cols[:, g : g + 1]
                            )
                            nc.vector.tensor_scalar_mul(
                                hb2[:, g, :], hx[:, g, :], b2cols[:, g : g + 1]
                            )

                        AHEAD = 3  # megatiles of lhs-table lead over main
                        for g in range(mt * AHEAD):
                            emit_hbuild(g)
                        lhs_dummy = hb1[:, 0, :]
                        mm(acc1, lhs_dummy, zrow, 0, rows, start=True)
                        mm(acc2, lhs_dummy, zrow, 0, rows, start=True)
                        mm(acc3, lhs_dummy, zrow, 0, rows, start=True)
                        for m in range(nm):
                            for g in range(
                                mt * (m + AHEAD), min(mt * (m + AHEAD + 1), ng)
                            ):
                                emit_hbuild(g)
                            adjt_t = adjts[m]
                            for t in range(mt):
                                g = mt * m + t
                                at = adjt_t[:, t * rows : (t + 1) * rows]
                                s, e = sg[g], eg[g]
                                if e > s:
                                    q = mixp.tile([128, mixmax], BF16)
                                    nc.vector._custom_dve(
                                        brmax,
                                        out=q[:, 0 : e - s],
                                        in0=alpha1bc[:, s:e],
                                        in1=alpha2bc[:, s:e],
                                        s0=b1cols[:, g : g + 1],
                                        s1=b2cols[:, g : g + 1],
                                    )
                                    qm = mixp.tile([128, mixmax], BF16)
                                    nc.vector.tensor_tensor(
                                        qm[:, 0 : e - s],
                                        q[:, 0 : e - s],
                                        at[:, s:e],
                                        OP.mult,
                                    )
                                if s > 0:
                                    mm(acc1, hb1[:, g, :], at, 0, s)
                                if e < rows:
                                    mm(acc2, hb2[:, g, :], at, e, rows)
                                if e > s:
                                    for c0, c1 in spans(s, e):
                                        nc.tensor.matmul(
                                            acc3[:, c0:c1],
                                            hx[:, g, :],
                                            qm[:, c0 - s : c1 - s],
                                            start=False,
                                            stop=False,
                                        )
                        mm(acc1, lhs_dummy, zrow, 0, 8, stop=True)
                        mm(acc2, lhs_dummy, zrow, 0, 8, stop=True)
                        mm(acc3, lhs_dummy, zrow, 0, 8, stop=True)

                    # ---- tail, pipelined per 128-col chunk ----
                    # numden = a1*acc1 + a2*acc2 + acc3; y = elu(num/den)
                    with tc.tile_pool(
                        name="pstail", bufs=2, space="PSUM"
                    ) as pstail:
                        for cc in range(rows // 128):
                            sl = slice(cc * 128, (cc + 1) * 128)
                            t1 = tailp.tile([de, 128], F32)
                            nc.vector.tensor_tensor(
                                t1, acc1[:, sl], alpha1bc[0:de, sl], OP.mult
                            )
                            t2 = tailp.tile([de, 128], F32)
                            nc.vector.tensor_tensor(
                                t2, acc2[:, sl], alpha2bc[0:de, sl], OP.mult
                            )
                            nc.vector.tensor_tensor(t1, t1, t2, OP.add)
                            nc.vector.tensor_tensor(
                                t1, t1, acc3[:, sl], OP.add
                            )
                            nc.vector.reciprocal(
                                t1[dout:de, :], t1[dout:de, :]
                            )
                            tp = pstail.tile([128, de], F32)
                            nc.tensor.transpose(
                                tp, t1, identity[0:de, 0:de]
                            )
                            # elu(v) = max(v, exp(min(v,0)) - 1), v = hp/s
                            vm = tailp.tile([128, dout], F32)
                            nc.vector.tensor_scalar(
                                vm, tp[:, 0:dout], tp[:, dout:de], 0.0,
                                OP.mult, OP.min,
                            )
                            e2 = tailp.tile([128, dout], F32)
                            nc.scalar.activation(e2, vm, AF.Exp)
                            ysb = tailp.tile([128, dout], F32)
                            nc.vector._custom_dve(
                                emx, out=ysb, in0=tp[:, 0:dout], in1=e2,
                                s0=tp[:, dout:de], s1=0.0, imm2=0.0,
                            )
                            nc.sync.dma_start(
                                out=y_d[cc * 128 : (cc + 1) * 128, :], in_=ysb
                            )
    nc.compile()
    return nc


def _run(x, adj, w, a, a_coeff, b_coeff, c_coeff, d_coeff):
    global LAST_RESULTS, LAST_NC
    n, din = x.shape
    dout = w.shape[1]
    assert adj.shape == (n, n) and a.shape == (2 * dout, 1)
    rows = n // NCORES

    A = float(np.asarray(a_coeff).reshape(-1)[0])
    B = float(np.asarray(b_coeff).reshape(-1)[0])
    C = float(np.asarray(c_coeff).reshape(-1)[0])
    D0 = float(np.asarray(d_coeff).reshape(-1)[0])
    L1 = _leaky(A + B)
    assert L1 >= 0.0 and C > 0.0, "branch-split kernel assumes L1>=0, C>0"
    kpre = L1 * C

    x = np.ascontiguousarray(x, dtype=np.float32)
    adj = np.asarray(adj, dtype=np.float32)
    assert ((adj == 0.0) | (adj == 1.0)).all(), "adj must be binary"
    w = np.ascontiguousarray(w, dtype=np.float32)
    a = np.ascontiguousarray(a, dtype=np.float32)

    # host-derived sort + range tables (layout prep, same category as the
    # baseline's G shift)
    h = x @ w
    hz1 = h @ a[:dout, 0]
    hz2 = h @ a[dout:, 0]
    cand = []
    for u in (hz1.min(), hz1.max()):
        for v in (hz2.min(), hz2.max()):
            cand.append(L1 * _leaky(C * (float(u) + float(v)) + D0))
    G = float(max(cand))
    M1 = float(kpre * hz1.max())
    M2 = float(SLOPE * kpre * hz1.max())
    # alpha1 = exp(kpre*hz1 + c_a1) etc.
    c_a1 = -M1
    c_a2 = -M2
    c_b1 = L1 * D0 - G + M1
    c_b2 = SLOPE * L1 * D0 - G + M2

    perm_j = np.argsort(hz2, kind="stable")
    perm_i = np.argsort(-hz1, kind="stable")
    own = [perm_i[c::NCORES] for c in range(NCORES)]
    tau = -hz1 - D0 / C  # z>=0  <=>  hz2_j >= tau_i
    hz2s = hz2[perm_j]
    ng = n // 128
    PAD = 5e-3
    sg = np.zeros(ng, dtype=int)
    eg = np.zeros(ng, dtype=int)
    for g in range(ng):
        lo = hz2s[g * 128] - PAD
        hi = hz2s[g * 128 + 127] + PAD
        sg[g] = min(
            np.searchsorted(tau[own[c]], lo, side="right") for c in range(NCORES)
        )
        eg[g] = max(
            np.searchsorted(tau[own[c]], hi, side="right") for c in range(NCORES)
        )
    # 8-align ranges: fp8 matmul moving operands need aligned element counts
    sg = (sg // 8) * 8
    eg = np.minimum(((eg + 7) // 8) * 8, rows)
    assert np.all(np.diff(sg) >= 0) and np.all(np.diff(eg) >= 0)
    mixmax = int(max(1, (eg - sg).max()))
    assert mixmax <= 256, f"mixed region too wide: {mixmax}"

    nc = _build(
        n, din, dout, rows, kpre, sg.tolist(), eg.tolist(), mixmax,
        c_a1, c_a2, c_b1, c_b2,
    )
    LAST_NC = nc

    xt_s = np.ascontiguousarray(x.T[:, perm_j]).astype(np.float16)
    in_maps = []
    for c in range(NCORES):
        o = own[c]
        adjt = np.ascontiguousarray(
            adj[o][:, perm_j].T.astype(ml_dtypes.float8_e5m2)
        )
        in_maps.append(
            {
                "adjt": adjt,
                "xt": xt_s,
                "xt_own": np.ascontiguousarray(x.T[:, o]).astype(np.float16),
                "w": w,
                "a": a,
            }
        )

    res = run_bass_kernel_spmd(
        nc, in_maps, core_ids=list(range(NCORES)), trace=TRACE
    )
    LAST_RESULTS = res
    ys = np.empty((n, dout), dtype=np.float32)
    for c in range(NCORES):
        ys[own[c]] = res.results[c]["y"]
    return ys


def kernel(x, adj, w, a, a_coeff, b_coeff, c_coeff, d_coeff):
    return _run(x, adj, w, a, a_coeff, b_coeff, c_coeff, d_coeff)
